# revision 5
# baseline (speedup 1.0000x reference)
"""Trainium2 Bass kernel for a 2-layer GCN (NextHopGNN) — v2.

Distribution: destination-node sharding across 8 NeuronCores (each core owns
npc=12500 dst nodes and all edges into them). Self-loops via identity matmul,
bias via rank-1 sqrt(deg) matmul (both exact, as in v1).

v2 changes vs v1:
- Table stored 4-node-packed bf16: row j of y_full holds nodes 4j..4j+3
  (64 features each, 256 bf16 = 512B). Gather descriptors are 512B (no
  small-transfer penalty), gather indices are pos>>2 < 25088 (single int16
  space, no src blocks), AllGathers move half the bytes.
- Buckets per layer: (psum-window[2] x src-parity[4]) = 8 instead of
  (dst_tile[98] x src_block[4]) = 392 -> padding ~1% instead of ~18%.
- Within a bucket, edges sorted by dst; each 128-edge chunk spans <=2 dst
  tiles; the 49 window accumulators live in PSUM across the whole window.
- bf16 matmuls everywhere (S one-hot bf16, gathered rows bf16).
"""
import sys
import numpy as np

sys.path.insert(0, "/opt/trn_rl_repo")

P = 128
H = 64
EDIM = 128
NCORES = 8
GCH = 8          # chunks (128 idx) per dma_gather instruction
SBATCH = 8       # chunks per batched one-hot build
WTILES = 32      # dst tiles per PSUM window (4 banks x 8 accs)
PACK = 4         # nodes packed per table row
USE_PREP = False # prepare_only + trigger_dma gathers

_COMPILED = {}


def _ceil_div(a, b):
    return (a + b - 1) // b


def make_schedule(edge_index, n_nodes, n_cores=NCORES):
    """Shard edges by dst core; bucket by (window, src-parity); sort by dst;
    chunk to 128 with shared (max-over-core) chunk counts."""
    src = np.asarray(edge_index[0], dtype=np.int64)
    dst = np.asarray(edge_index[1], dtype=np.int64)
    npc = n_nodes // n_cores
    T = _ceil_div(npc, P)              # 98
    rpc = T * P                        # ranks per core (12544)
    NW = 1                             # single window (SBUF accumulators)
    NPOS = n_cores * rpc               # position space (100352)
    NPACK = NPOS // PACK               # packed table rows (25088)

    # position of a global node id
    def pos_of(g):
        c = g // npc
        return c * rpc + (g - c * npc)

    pos_src = pos_of(src)

    counts = np.zeros((n_cores, NW, PACK), np.int64)
    percore = []
    for c in range(n_cores):
        sel = (dst >= c * npc) & (dst < (c + 1) * npc)
        s = pos_src[sel]
        d = dst[sel] - c * npc
        t = d >> 7
        w = np.zeros_like(t)
        par = s & (PACK - 1)
        key = (w * PACK + par) * (1 << 32) + d
        order = np.argsort(key, kind="stable")
        s, d, w, par = s[order], d[order], w[order], par[order]
        cnt = np.zeros((NW, PACK), np.int64)
        for wi in range(NW):
            for pi in range(PACK):
                cnt[wi, pi] = int(((w == wi) & (par == pi)).sum())
        counts[c] = cnt
        percore.append((s, d, cnt))

    K = np.maximum(_ceil_div(counts.max(axis=0), P), 1)  # [NW, PACK] chunks
    total_chunks = int(K.sum())

    # chunk-order bookkeeping: buckets processed w-major then parity
    m_start = np.zeros((NW, PACK), np.int64)
    m = 0
    for wi in range(NW):
        for pi in range(PACK):
            m_start[wi, pi] = m
            m += K[wi, pi]

    # gather groups per bucket (instruction = up to GCH chunks)
    gather_groups = []  # list of (w, par, chunk_start_in_bucket, n_chunks)
    for wi in range(NW):
        for pi in range(PACK):
            rem = int(K[wi, pi])
            st = 0
            while rem > 0:
                g = min(GCH, rem)
                gather_groups.append((wi, pi, st, g))
                st += g
                rem -= g

    # per-chunk matmul plan (same for all cores by construction? No - the
    # dst composition differs per core. The PROGRAM must be identical, so
    # the matmul plan must be shared: use the union over cores of tiles
    # spanned by each chunk.)
    # chunk m covers edges [m*128, (m+1)*128) of its bucket stream; tiles
    # spanned differ per core -> take union over cores.
    chunk_tiles = [set() for _ in range(total_chunks)]
    per_core_streams = []
    for c in range(n_cores):
        s, d, cnt = percore[c]
        # bucket start offsets in the sorted arrays
        off = np.zeros((NW, PACK), np.int64)
        o = 0
        for wi in range(NW):
            for pi in range(PACK):
                off[wi, pi] = o
                o += cnt[wi, pi]
        idx_stream = np.zeros(total_chunks * P, np.int16)
        dst_stream = np.full(total_chunks * P, -1.0, np.float32)
        for wi in range(NW):
            for pi in range(PACK):
                n = int(cnt[wi, pi])
                if n == 0:
                    continue
                o0 = int(off[wi, pi])
                ss = s[o0:o0 + n]
                dd = d[o0:o0 + n]
                q0 = int(m_start[wi, pi]) * P
                idx_stream[q0:q0 + n] = (ss >> 2).astype(np.int16)
                dst_stream[q0:q0 + n] = dd.astype(np.float32)
                for k in range(_ceil_div(n, P)):
                    mloc = int(m_start[wi, pi]) + k
                    tl = dd[k * P:(k + 1) * P] >> 7
                    for t in np.unique(tl):
                        chunk_tiles[mloc].add(int(t))
        per_core_streams.append((idx_stream, dst_stream))

    # rebase dst_stream per chunk to its first tile; build matmul plan
    plan = []  # per chunk: list of (tile, variant) variant 0: iota0, 1: iota+128
    base_tile = np.zeros(total_chunks, np.int64)
    for mloc in range(total_chunks):
        tls = sorted(chunk_tiles[mloc])
        if not tls:
            tls = [T - 1]
        t0 = tls[0]
        base_tile[mloc] = t0
        assert tls[-1] - t0 <= 5, (mloc, tls)
        plan.append([(t, t - t0) for t in tls])

    core_data = []
    for c in range(n_cores):
        idx_stream, dst_stream = per_core_streams[c]
        dst_loc = dst_stream.reshape(total_chunks, P)
        dst_loc = dst_loc - (base_tile[:, None] * P) * (dst_loc >= 0)
        # pad slots keep -1 (matches no iota value in [0,256))
        idx_wrapped = idx_stream.reshape(-1, 16).T       # [16, total_chunks*8]
        idx_wrapped = np.tile(idx_wrapped, (8, 1)).astype(np.int16)
        dst_t = np.ascontiguousarray(dst_loc.T).astype(np.float32)  # [128, chunks]
        core_data.append((idx_wrapped, dst_t))
    # dstloc values are small ints (-129..255): exact in bf16

    # runs: per parity bucket, maximal consecutive chunk groups per tile
    runs = []  # list of (par, tile, [(chunk, var), ...]) in stream order
    for pi in range(PACK):
        lo = int(m_start[0, pi])
        hi = lo + int(K[0, pi])
        open_t = {}
        order = []
        for m in range(lo, hi):
            for (t, var) in plan[m]:
                if t not in open_t:
                    open_t[t] = []
                    order.append(t)
                open_t[t].append((m, var))
        for t in order:
            runs.append((pi, t, open_t[t]))

    return {
        "n_nodes": n_nodes, "n_cores": n_cores, "npc": npc, "T": T,
        "rpc": rpc, "NW": NW, "NPOS": NPOS, "NPACK": NPACK,
        "K": K, "total_chunks": total_chunks, "m_start": m_start,
        "gather_groups": gather_groups, "plan": plan, "runs": runs,
        "base_tile": base_tile, "core_data": core_data,
    }


def build_bass(sched):
    from concourse import bass, bacc, tile, mybir
    from concourse.masks import make_identity

    n_cores = sched["n_cores"]
    npc = sched["npc"]
    T = sched["T"]
    rpc = sched["rpc"]
    NW = sched["NW"]
    NPACK = sched["NPACK"]
    K = sched["K"]
    total_chunks = sched["total_chunks"]
    m_start = sched["m_start"]
    gather_groups = sched["gather_groups"]
    plan = sched["plan"]
    f32 = mybir.dt.float32
    bf16 = mybir.dt.bfloat16
    i16 = mybir.dt.int16
    i32 = mybir.dt.int32

    nc = bacc.Bacc("TRN2", target_bir_lowering=False, debug=False,
                   enable_asserts=True, num_devices=n_cores)

    embT = nc.dram_tensor("embT", [P, T * P], bf16, kind="ExternalInput")
    W1_d = nc.dram_tensor("W1", [EDIM, H], bf16, kind="ExternalInput")
    W2_d = nc.dram_tensor("W2", [H, H], bf16, kind="ExternalInput")
    b1_d = nc.dram_tensor("b1r", [1, H], f32, kind="ExternalInput")
    b2_d = nc.dram_tensor("b2r", [1, H], f32, kind="ExternalInput")
    dinv_d = nc.dram_tensor("dinv_t", [P, T], f32, kind="ExternalInput")
    sqd_d = nc.dram_tensor("sqd_row", [1, T * P], f32, kind="ExternalInput")
    idx_d = nc.dram_tensor("idxs", [P, total_chunks * 8], i16, kind="ExternalInput")
    dst_d = nc.dram_tensor("dstloc", [P, total_chunks], f32, kind="ExternalInput")
    out_d = nc.dram_tensor("out", [npc, H], f32, kind="ExternalOutput")

    # packed tables: row j holds nodes 4j..4j+3 (pos space), 256 bf16 = 512B
    rows_pc = rpc // PACK                      # 3136 rows per core slice
    y1_in = nc.dram_tensor("y1_in", [rows_pc, PACK * H], bf16)
    y1_full = nc.dram_tensor("y1_full", [NPACK, PACK * H], bf16,
                             addr_space="Shared")
    y2_in = nc.dram_tensor("y2_in", [rows_pc, PACK * H], bf16)
    y2_full = nc.dram_tensor("y2_full", [NPACK, PACK * H], bf16,
                             addr_space="Shared")

    with tile.TileContext(nc) as tc:
        with tc.tile_pool(name="const", bufs=1) as constp, \
             tc.tile_pool(name="tables", bufs=1) as tablep, \
             tc.tile_pool(name="work", bufs=3) as workp, \
             tc.tile_pool(name="gath", bufs=6) as gathp, \
             tc.tile_pool(name="spool", bufs=3) as spool, \
             tc.tile_pool(name="psB", bufs=1, space="PSUM") as psB:

            ident = constp.tile([P, P], bf16)
            make_identity(nc, ident[:])
            iota_i = constp.tile([P, SBATCH * P], i32)
            nc.gpsimd.iota(iota_i[:], pattern=[[0, SBATCH], [1, P]],
                           base=0, channel_multiplier=0)
            iota0 = constp.tile([P, SBATCH * P], f32)
            nc.vector.tensor_copy(iota0[:], iota_i[:])
            max_var = max((v for pl in plan for (_, v) in pl), default=0)
            iotas = [iota0]
            for v in range(1, max_var + 1):
                iv_i = constp.tile([P, P], i32, name=f"ivi{v}")
                nc.gpsimd.iota(iv_i[:], pattern=[[1, P]], base=128 * v,
                               channel_multiplier=0)
                iv = constp.tile([P, P], f32, name=f"iv{v}")
                nc.vector.tensor_copy(iv[:], iv_i[:])
                iotas.append(iv)
            iota1w_i = constp.tile([P, SBATCH * P], i32, name="iota1wi")
            nc.gpsimd.iota(iota1w_i[:], pattern=[[0, SBATCH], [1, P]],
                           base=128, channel_multiplier=0)
            iota1w = constp.tile([P, SBATCH * P], f32, name="iota1w")
            nc.vector.tensor_copy(iota1w[:], iota1w_i[:])
            iotas2 = [iota0, iota1w]

            W1_s = constp.tile([EDIM, H], bf16)
            nc.sync.dma_start(out=W1_s[:], in_=W1_d[:])
            W2_s = constp.tile([H, H], bf16)
            nc.sync.dma_start(out=W2_s[:], in_=W2_d[:])
            b1_s = constp.tile([1, H], f32)
            nc.sync.dma_start(out=b1_s[:], in_=b1_d[:])
            b2_s = constp.tile([1, H], f32)
            nc.sync.dma_start(out=b2_s[:], in_=b2_d[:])
            dinv_s = constp.tile([P, T], f32)
            nc.sync.dma_start(out=dinv_s[:], in_=dinv_d[:])
            sqd_s = constp.tile([1, T * P], f32)
            nc.sync.dma_start(out=sqd_s[:], in_=sqd_d[:])

            y1_all = tablep.tile([P, T * H], bf16)
            y2_all = tablep.tile([P, T * H], bf16)

            def pack_view(dram_t, t):
                # [128, 64] view of packed rows for dst tile t
                return dram_t[t * (P // PACK):(t + 1) * (P // PACK), :] \
                    .rearrange("q (h f) -> (q h) f", h=PACK)

            # ---- phase 1: y1 = dinv * (emb @ W1) ----
            for t in range(T):
                xt = workp.tile([P, P], bf16, tag="embT")
                nc.sync.dma_start(out=xt[:], in_=embT[:, t * P:(t + 1) * P])
                ps = psB.tile([P, H], f32, tag="ps")
                nc.tensor.matmul(ps[:], lhsT=xt[:], rhs=W1_s[:],
                                 start=True, stop=True)
                ys = y1_all[:, t * H:(t + 1) * H]
                nc.vector.tensor_scalar_mul(ys, ps[:], dinv_s[:, t:t + 1])
                nc.sync.dma_start(out=pack_view(y1_in, t), in_=ys)

            # ---- phase 2: AllGather layer-1 table ----
            nc.gpsimd.collective_compute(
                "AllGather", mybir.AluOpType.bypass,
                replica_groups=[list(range(n_cores))],
                ins=[y1_in[:]],
                outs=[y1_full[:]],
            )

            # ---- aggregation (both layers) ----
            def aggregation(full_table, y_own, b_s, layer):
                runs = sched["runs"]
                gidx = [0]
                gbufs = {}
                sbuf = {}
                dst_tile = [None, -1]

                sacc = tablep.tile([P, T * H], f32, name=f"sacc{layer}")
                first_write = {}

                def ensure_gather(pi, k):
                    def find(pi, k):
                        for (gt, st, ng) in gbufs.get(pi, []):
                            if st <= k < st + ng:
                                return gt[:, k - st, :]
                        return None
                    view = find(pi, k)
                    while view is None:
                        gw, gp_, st, ng = gather_groups[gidx[0]]
                        gidx[0] += 1
                        mm0 = int(m_start[gw, gp_]) + st
                        it = workp.tile([P, GCH * 8], i16, tag="idx",
                                        name="idxt", bufs=6)
                        nc.scalar.dma_start(
                            out=it[:, :ng * 8],
                            in_=idx_d[:, mm0 * 8:(mm0 + ng) * 8])
                        gt = gathp.tile([P, GCH, PACK * H], bf16,
                                        tag=f"g{layer}", name="gt")
                        nc.gpsimd.dma_gather(
                            out_ap=gt[:, :ng, :],
                            in_ap=full_table[:],
                            idxs_ap=it[:, :ng * 8],
                            num_idxs=ng * P,
                            num_idxs_reg=ng * P,
                            elem_size=PACK * H,
                        )
                        gbufs.setdefault(gp_, []).append((gt, st, ng))
                        if len(gbufs[gp_]) > 2:
                            gbufs[gp_].pop(0)
                        view = find(pi, k)
                    return view

                DSTW = 64

                def ensure_dst(lo):
                    dlo = (lo // DSTW) * DSTW
                    if dst_tile[1] != dlo:
                        dt_ = workp.tile([P, DSTW], f32, tag="dst",
                                         name="dstt", bufs=3)
                        w_ = min(DSTW, total_chunks - dlo)
                        nc.scalar.dma_start(out=dt_[:, :w_],
                                            in_=dst_d[:, dlo:dlo + w_])
                        dst_tile[0] = dt_
                        dst_tile[1] = dlo
                    return dst_tile[0], dlo

                def build_batch(lo):
                    # batched one-hot for vars 0 and 1 over SBATCH chunks
                    dtile, dlo = ensure_dst(lo)
                    nb = min(SBATCH, total_chunks - lo)
                    col = lo - dlo
                    dl3 = dtile[:, col:col + nb].rearrange(
                        "p (c u) -> p c u", u=1)
                    sts = []
                    for var in (0, 1):
                        stv = spool.tile([P, SBATCH * P], bf16,
                                         tag=f"S{var}", name=f"st{var}")
                        nc.vector.tensor_tensor(
                            out=stv[:, :nb * P].rearrange(
                                "p (c j) -> p c j", j=P),
                            in0=iotas2[var][:, :nb * P].rearrange(
                                "p (c j) -> p c j", j=P),
                            in1=dl3.to_broadcast([P, nb, P]),
                            op=mybir.AluOpType.is_equal)
                        sts.append(stv)
                    sbuf.clear()
                    sbuf[lo] = sts

                def ensure_s(mloc, var=0):
                    lo = (mloc // SBATCH) * SBATCH
                    if lo not in sbuf:
                        build_batch(lo)
                    return sbuf[lo][var][:, (mloc - lo) * P:
                                         (mloc - lo + 1) * P]

                def s_variant(mloc, var):
                    if var <= 1:
                        return ensure_s(mloc, var)
                    lo = (mloc // SBATCH) * SBATCH
                    dtile, dlo = ensure_dst(lo)
                    st1 = spool.tile([P, P], bf16, tag="Sx", name="st1")
                    col = mloc - dlo
                    dl3 = dtile[:, col:col + 1].rearrange(
                        "p (c u) -> p c u", u=1)
                    nc.vector.tensor_tensor(
                        out=st1[:].rearrange("p (c j) -> p c j", j=P),
                        in0=iotas[var][:, :P].rearrange(
                            "p (c j) -> p c j", j=P),
                        in1=dl3.to_broadcast([P, 1, P]),
                        op=mybir.AluOpType.is_equal)
                    return st1[:]

                remaining = {}
                for (_, t, _mvs) in runs:
                    remaining[t] = remaining.get(t, 0) + 1

                def finish_tile(t):
                    # self-loop + bias, then hand to the epilogue
                    psr2 = psB.tile([P, H], f32, tag="run", name="psr2",
                                    bufs=4)
                    nc.tensor.matmul(psr2[:], lhsT=ident[:],
                                     rhs=y_own[:, t * H:(t + 1) * H],
                                     start=True, stop=False)
                    nc.tensor.matmul(psr2[:],
                                     lhsT=sqd_s[:, t * P:(t + 1) * P],
                                     rhs=b_s[:], start=False, stop=True)
                    sl = sacc[:, t * H:(t + 1) * H]
                    if t not in first_write:
                        nc.vector.tensor_copy(sl, psr2[:])
                        first_write[t] = True
                    else:
                        nc.vector.tensor_tensor(
                            out=sl, in0=sl, in1=psr2[:],
                            op=mybir.AluOpType.add)
                    return sl

                for (pi, t, mvs) in runs:
                    psr = psB.tile([P, H], f32, tag="run", name="psr",
                                   bufs=4)
                    for j, (mloc, var) in enumerate(mvs):
                        k = mloc - int(m_start[0, pi])
                        gview = ensure_gather(pi, k)
                        rhs = gview.rearrange(
                            "p (q h) -> p q h", h=H)[:, pi, :]
                        sview = ensure_s(mloc, 0) if var == 0 \
                            else s_variant(mloc, var)
                        nc.tensor.matmul(psr[:], lhsT=sview, rhs=rhs,
                                         start=(j == 0),
                                         stop=(j == len(mvs) - 1))
                    sl = sacc[:, t * H:(t + 1) * H]
                    if t not in first_write:
                        nc.vector.tensor_copy(sl, psr[:])
                        first_write[t] = True
                    else:
                        nc.vector.tensor_tensor(
                            out=sl, in0=sl, in1=psr[:],
                            op=mybir.AluOpType.add)
                    remaining[t] -= 1
                    if remaining[t] == 0:
                        yield t, finish_tile(t)

                for t in range(T):
                    if t not in remaining:
                        yield t, finish_tile(t)

            # ---- phase 3: layer-1 aggregation + fused layer-2 table ----
            for t, ps in aggregation(y1_full, y1_all, b1_s, 1):
                h1 = workp.tile([P, H], bf16, tag="h1")
                nc.scalar.activation(h1[:], ps,
                                     mybir.ActivationFunctionType.Relu,
                                     scale=dinv_s[:, t:t + 1])
                pT = psB.tile([H, P], bf16, tag="pT")
                nc.tensor.transpose(pT[:], h1[:], ident[:])
                h1T = workp.tile([H, P], bf16, tag="h1T")
                nc.vector.tensor_copy(h1T[:], pT[:])
                ps2 = psB.tile([P, H], f32, tag="ps2")
                nc.tensor.matmul(ps2[:], lhsT=h1T[:], rhs=W2_s[:],
                                 start=True, stop=True)
                y2s = y2_all[:, t * H:(t + 1) * H]
                nc.vector.tensor_scalar_mul(y2s, ps2[:], dinv_s[:, t:t + 1])
                nc.sync.dma_start(out=pack_view(y2_in, t), in_=y2s)

            # ---- phase 4: AllGather layer-2 table ----
            nc.gpsimd.collective_compute(
                "AllGather", mybir.AluOpType.bypass,
                replica_groups=[list(range(n_cores))],
                ins=[y2_in[:]],
                outs=[y2_full[:]],
            )

            # ---- phase 5: layer-2 aggregation -> output ----
            for t, ps in aggregation(y2_full, y2_all, b2_s, 2):
                ot = workp.tile([P, H], f32, tag="ot")
                nc.scalar.activation(ot[:], ps,
                                     mybir.ActivationFunctionType.Copy,
                                     scale=dinv_s[:, t:t + 1])
                rows = min(npc - t * P, P)
                nc.sync.dma_start(out=out_d[t * P:t * P + rows, :],
                                  in_=ot[:rows, :])

    nc.compile()
    return nc


def make_inputs(sched, emb_weight, W1, b1, W2, b2, deg):
    import ml_dtypes
    n_cores = sched["n_cores"]
    npc = sched["npc"]
    T = sched["T"]
    dinv = (1.0 / np.sqrt(deg.astype(np.float64))).astype(np.float32)
    sqd = np.sqrt(deg.astype(np.float64)).astype(np.float32)
    in_maps = []
    for c in range(n_cores):
        lo, hi = c * npc, (c + 1) * npc
        embT = np.zeros((P, T * P), ml_dtypes.bfloat16)
        embT[:, :npc] = emb_weight[lo:hi].astype(ml_dtypes.bfloat16).T
        tmp = np.zeros(T * P, np.float32)
        tmp[:npc] = dinv[lo:hi]
        dinv_t = np.ascontiguousarray(tmp.reshape(T, P).T)
        sqd_row = np.zeros((1, T * P), np.float32)
        sqd_row[0, :npc] = sqd[lo:hi]
        idx_wrapped, dst_t = sched["core_data"][c]
        m = {
            "embT": embT,
            "W1": W1.astype(ml_dtypes.bfloat16),
            "W2": W2.astype(ml_dtypes.bfloat16),
            "b1r": b1.reshape(1, -1).astype(np.float32),
            "b2r": b2.reshape(1, -1).astype(np.float32),
            "dinv_t": dinv_t,
            "sqd_row": sqd_row,
            "idxs": idx_wrapped,
            "dstloc": dst_t,
        }
        in_maps.append(m)
    return in_maps


def run(edge_index, emb_weight, W1, b1, W2, b2, n_nodes=None, trace=False):
    from concourse import bass_utils
    n_nodes = n_nodes if n_nodes is not None else emb_weight.shape[0]
    sched = make_schedule(np.asarray(edge_index), n_nodes)
    key = ("gnnv2", n_nodes, int(sched["total_chunks"]),
           tuple(tuple(int(x) for x in row) for row in sched["K"]),
           tuple(tuple(p) for p in
                 (tuple((t, v) for t, v in pl) for pl in sched["plan"])))
    kh = hash(key)
    if kh not in _COMPILED:
        _COMPILED[kh] = build_bass(sched)
    nc = _COMPILED[kh]
    deg = np.bincount(np.asarray(edge_index)[1],
                      minlength=n_nodes).astype(np.float32) + 1.0
    in_maps = make_inputs(sched, np.asarray(emb_weight), np.asarray(W1),
                          np.asarray(b1), np.asarray(W2), np.asarray(b2), deg)
    res = bass_utils.run_bass_kernel_spmd(
        nc, in_maps, core_ids=list(range(sched["n_cores"])), trace=trace)
    npc = sched["npc"]
    out = np.concatenate([res.results[c]["out"]
                          for c in range(sched["n_cores"])], axis=0)
    return out[:n_nodes], res


def kernel(edge_index, emb_weight, W1, b1, W2, b2):
    out, _ = run(edge_index, emb_weight, W1, b1, W2, b2)
    return out
